# revision 16
# baseline (speedup 1.0000x reference)
"""Trainium2 Bass kernel for nn_BbVertLoss (point-in-bbox CE + IoU + L2 loss).

Strategy (pure data parallel, hardcoded for B=16, N=40960, H=24, 8 cores):
  - Each core gets 2 batches. Points live on partitions: partition p holds
    640 consecutive points of batch p//64 (local), laid out [128, 640*9].
  - Pred leg (real units, clip needs them):
      u_d = r_d^2 - (x_d-c_d)^2;  tcl = clip(max_d((x_d-c_d)^2 - r_d^2), +-.2)
      p = sigmoid(-100*tcl), accum -> S_p
  - GT leg (normalized units, only the sign matters):
      q_d = ((x_d-c'_d)/r'_d)^2 computed as Square(x*s + t) on ACT
      (per-partition scale AP!), combined with a stock bf16 TT MAX (2x mode),
      then one fused DVE op: g = (max(sq(x*s_x+t_x), m_yz) < 1), accum -> S_g
  - sel' = helper*|p + g - 1| + (1-helper)  (helper folded in so the Ln
    accumulation can batch across h in one group-wide instruction whose
    single accumulator directly yields the helper-weighted CE partial).
  - Ln per group of GRP h: one ACT op over [128, GRP*640], accum -> CE.
  - Host: partition+core reduction and final combine in f64.
  - Custom fused DVE ops registered into concourse.dve_ops:
      ANT_SUB2MAX:     max(in0-s0, in1-s1)
      ANT_SQMAXCLIP:   clip(max((in0-s0)^2-s1, in1), imm2, -imm2)
      ANT_SQMAXLT1SUM: (max((in0*s0+s1)^2, in1) < 1) [+ sum]
      ANT_SELHSUM:     s0*|in0 + (in1-1)| + s1 [+ sum]
      ANT_SQSUB / ANT_SQSUBMAX: DVE-route pred legs for engine balance
"""

import numpy as np

B, N, H = 16, 40960, 24
NCORES = 8
BPC = B // NCORES            # batches per core = 2
PPB = 64                     # partitions per batch
FPT = N // PPB               # points per partition = 640
NPART = BPC * PPB            # 128
RAWF = FPT * 3               # xyz de-interleaved on host: [x|y|z] per partition
import os
GRP = int(os.environ.get("KGRP", "8"))   # h-group size (ACT table amortization)
WBUFS = int(os.environ.get("KWBUFS", "4"))
NB = int(os.environ.get("KNB", "10"))     # of 24 h: pred legs on DVE route
SCW = 16                     # scal columns per (batch, h)
NGRP = (H + GRP - 1) // GRP

_CACHE = {}


def _register_custom_ops():
    """Register fused DVE ops in the module-level registries (idempotent)."""
    import concourse.dve_ops as dops
    from concourse.dve_spec import (Spec, Src0, Src1, C0, C1, C2, Zero, One,
                                    maxx, minn, sq, lower, AluOp)
    from concourse.dve_table_gen import dve_ver_for
    from concourse.dve_uop import DveOpSpec

    if "ANT_SUB2MAX" in dops._SUB_OPCODE_FOR_NAME:
        _CACHE["ops"] = {o.name: o for o in dops.OPS}
        return

    ver = dve_ver_for("TRN2")

    def ref_sub2max(in0, in1, s0, s1, imm2):
        return np.maximum(in0 - s0, in1 - s1)

    def ref_sqmaxclip(in0, in1, s0, s1, imm2):
        return np.minimum(np.maximum(np.maximum((in0 - s0) ** 2 - s1, in1),
                                     imm2), -imm2)

    def ref_sqmaxlt1sum(in0, in1, s0, s1, imm2):
        b = (np.maximum((in0 * s0 + s1) ** 2, in1) < 1.0).astype(np.float32)
        return b, b.reshape(b.shape[0], -1).sum(axis=-1, keepdims=True).astype(
            np.float32)

    def ref_selhsum(in0, in1, s0, s1, imm2):
        t = ((in1 - np.float32(1.0)) + in0).astype(np.float32)
        b = (s0 * np.abs(t) + s1).astype(np.float32)
        return b, b.reshape(b.shape[0], -1).sum(axis=-1, keepdims=True).astype(
            np.float32)

    def ref_sqsubmax(in0, in1, s0, s1, imm2):
        return np.maximum((in0 - s0) ** 2 - s1, in1)

    _t = Src0 + (Src1 - One)
    _q = sq(Src0 * C0 + C1)
    specs = [
        ("ANT_SUB2MAX", Spec(body=maxx(Src0 - C0, Src1 - C1),
                             reference=ref_sub2max)),
        ("ANT_SQMAXCLIP", Spec(body=minn(maxx(maxx(sq(Src0 - C0) - C1, Src1),
                                              C2), Zero - C2),
                               reference=ref_sqmaxclip)),
        ("ANT_SQMAXLT1SUM", Spec(body=(maxx(_q, Src1) < One),
                                 accum=AluOp.ADD, reference=ref_sqmaxlt1sum)),
        ("ANT_SELHSUM", Spec(body=maxx(_t, Zero - _t) * C0 + C1,
                             accum=AluOp.ADD, reference=ref_selhsum)),
        ("ANT_SQSUBMAX", Spec(body=maxx(sq(Src0 - C0) - C1, Src1),
                              reference=ref_sqsubmax)),
        ("ANT_SQSUB", Spec(body=sq(Src0 - C0) - C1,
                           reference=lambda in0, in1, s0, s1, imm2:
                               (in0 - s0) ** 2 - s1)),
    ]
    for name, spec in specs:
        opcode = max(dops._SUB_OPCODE_FOR_NAME.values()) + 1
        assert opcode < 0x20
        tmp = DveOpSpec(name=name, opcode=opcode, uops=lower(spec, ver=ver),
                        rd1_en=True)
        op = dops.DveOp(name, spec, subdim=False, uops_sha={ver: tmp.sha(ver)})
        dops.OPS.append(op)
        dops.CUSTOM_DVE_SPECS[name] = spec
        dops._SUB_OPCODE_FOR_NAME[name] = opcode
    _CACHE["ops"] = {o.name: o for o in dops.OPS}


def _build_module():
    import concourse.bacc as bacc
    import concourse.tile as tile
    from concourse import mybir

    _register_custom_ops()
    OPS = _CACHE["ops"]

    f32 = mybir.dt.float32
    bf16 = mybir.dt.bfloat16
    fp16 = mybir.dt.float16
    Act = mybir.ActivationFunctionType
    Alu = mybir.AluOpType

    # pred h's routed to all-DVE legs: the whole LAST group goes DVE so the
    # Scalar tail (last sigmoids + ln) shrinks while Vector's end-idle absorbs
    # it; the remainder spreads over the earlier h's.
    dve_hs = set(int(round(i * H / max(NB, 1))) for i in range(NB)) if NB else set()

    nc = bacc.Bacc("TRN2", debug=False)

    xpc = nc.dram_tensor("xpc", [NPART, RAWF], f32, kind="ExternalInput")
    scal = nc.dram_tensor("scal", [NPART, H * SCW], f32, kind="ExternalInput")
    accP_d = nc.dram_tensor("accP", [NPART, H], f32, kind="ExternalOutput")
    accG_d = nc.dram_tensor("accG", [NPART, H], f32, kind="ExternalOutput")
    accS_d = nc.dram_tensor("accS", [NPART, H], f32, kind="ExternalOutput")
    accL_d = nc.dram_tensor("accL", [NPART, 2 * NGRP], f32, kind="ExternalOutput")

    with tile.TileContext(nc) as tc:
        with (
            tc.tile_pool(name="data", bufs=1) as data,
            tc.tile_pool(name="work", bufs=WBUFS) as work,
            tc.tile_pool(name="phase", bufs=GRP + 3) as phase,
            tc.tile_pool(name="selp", bufs=2) as selp,
            tc.tile_pool(name="pp", bufs=4, space="PSUM") as pp,
        ):
            sc = data.tile([NPART, H * SCW], f32, tag="sc")
            nc.sync.dma_start(out=sc[:], in_=scal[:])
            raw = data.tile([NPART, RAWF], f32, tag="raw")
            for dord in (1, 2, 0):   # y first (consumed first), x last
                cs = slice(FPT * dord, FPT * (dord + 1))
                for half in range(2):
                    p0, p1 = 64 * half, 64 * (half + 1)
                    nc.sync.dma_start(out=raw[p0:p1, cs], in_=xpc[p0:p1, cs])
            eps8 = data.tile([NPART, 1], f32, tag="eps8")
            nc.vector.memset(eps8[:], 1e-8)
            warm = data.tile([NPART, 1], f32, tag="warm")
            nc.scalar.activation(warm[:], eps8[:], Act.Sigmoid,
                                 bias=0.0, scale=-100.0)

            accP = data.tile([NPART, H], f32, tag="accP")
            accG = data.tile([NPART, H], f32, tag="accG")
            accS = data.tile([NPART, H], f32, tag="accS")
            accL = data.tile([NPART, 2 * NGRP], f32, tag="accL")

            xs = [raw[:, FPT * d : FPT * (d + 1)] for d in range(3)]

            def col(h, j):
                return sc[:, SCW * h + j : SCW * h + j + 1]

            gsizes = [int(v) for v in os.environ.get(
                "KGL", "9,9,6").split(",")]
            assert sum(gsizes) == H
            gstarts = [sum(gsizes[:i]) for i in range(len(gsizes))]
            for gi, (h0, ng) in enumerate(zip(gstarts, gsizes)):
                hs = range(h0, h0 + ng)
                selbig = selp.tile([NPART, ng * FPT], bf16, tag="selbig")
                for h in hs:
                    # pred leg: tcl = clip(max_d((x_d-c_d)^2 - r_d^2), +-0.2)
                    if h in dve_hs:
                        qsy = work.tile([NPART, FPT], f32, tag="qsy")
                        nc.vector._custom_dve(OPS["ANT_SQSUB"], out=qsy[:],
                                              in0=xs[1],
                                              s0=col(h, 6), s1=col(h, 2))
                        m1 = work.tile([NPART, FPT], f32, tag="m1")
                        nc.vector._custom_dve(OPS["ANT_SQSUBMAX"], out=m1[:],
                                              in0=xs[2], in1=qsy[:],
                                              s0=col(h, 7), s1=col(h, 3))
                    else:
                        sqy = work.tile([NPART, FPT], f32, tag="sqy")
                        nc.scalar.activation(sqy[:], xs[1], Act.Square,
                                             bias=col(h, 0), scale=1.0)
                        sqz = work.tile([NPART, FPT], f32, tag="sqz")
                        nc.scalar.activation(sqz[:], xs[2], Act.Square,
                                             bias=col(h, 1), scale=1.0)
                        m1 = work.tile([NPART, FPT], f32, tag="m1")
                        nc.vector._custom_dve(OPS["ANT_SUB2MAX"], out=m1[:],
                                              in0=sqy[:], in1=sqz[:],
                                              s0=col(h, 2), s1=col(h, 3))
                    tcl = phase.tile([NPART, FPT], f32, tag="tcl")
                    nc.vector._custom_dve(OPS["ANT_SQMAXCLIP"], out=tcl[:],
                                          in0=xs[0], in1=m1[:],
                                          s0=col(h, 4), s1=col(h, 5), imm2=-0.2)

                    # gt leg, normalized: q_d = Square(x*s+t); in-box = all q<1
                    qgy = work.tile([NPART, FPT], bf16, tag="qgy")
                    nc.scalar.activation(qgy[:], xs[1], Act.Square,
                                         bias=col(h, 9), scale=col(h, 8))
                    qgz = work.tile([NPART, FPT], bf16, tag="qgz")
                    nc.scalar.activation(qgz[:], xs[2], Act.Square,
                                         bias=col(h, 11), scale=col(h, 10))
                    myz = work.tile([NPART, FPT], bf16, tag="myz")
                    nc.vector.tensor_tensor(out=myz[:], in0=qgy[:],
                                            in1=qgz[:], op=Alu.max)
                    g = phase.tile([NPART, FPT], bf16, tag="g")
                    nc.vector._custom_dve(OPS["ANT_SQMAXLT1SUM"], out=g[:],
                                          in0=xs[0], in1=myz[:],
                                          s0=col(h, 12), s1=col(h, 13),
                                          accum_out=accG[:, h : h + 1])
                    # tail fused per h: Square and Sigmoid share a
                    # table set, so interleaving costs no table loads and
                    # lets sel' tiles finish progressively before the ln
                    p = pp.tile([NPART, FPT], f32, tag="p")
                    nc.scalar.activation(p[:], tcl[:], Act.Sigmoid,
                                         bias=0.0, scale=-100.0,
                                         accum_out=accP[:, h : h + 1])
                    j = h - h0
                    nc.vector._custom_dve(
                        OPS["ANT_SELHSUM"],
                        out=selbig[:, j * FPT : (j + 1) * FPT],
                        in0=p[:], in1=g[:],
                        s0=col(h, 14), s1=col(h, 15),
                        accum_out=accS[:, h : h + 1])
                # one batched Ln for the whole group; its accumulator IS the
                # helper-weighted CE partial sum for these h
                nc.scalar.activation(selbig[:], selbig[:], Act.Ln,
                                     bias=eps8[:], scale=1.0,
                                     accum_out=accL[:, 2 * gi : 2 * gi + 1])

            nc.sync.dma_start(out=accP_d[:], in_=accP[:])
            nc.sync.dma_start(out=accG_d[:], in_=accG[:])
            nc.sync.dma_start(out=accS_d[:], in_=accS[:])
            nc.sync.dma_start(out=accL_d[:], in_=accL[:])

    nc.compile()
    return nc


def _get_module():
    if "nc" not in _CACHE:
        _CACHE["nc"] = _build_module()
    return _CACHE["nc"]


def _make_inputs(X_pc, y_bbvert_pred, Y_bbvert):
    """Build per-core input maps (host-side shard + scalar precompute)."""
    X_pc = np.ascontiguousarray(X_pc, dtype=np.float32)
    pred = np.asarray(y_bbvert_pred, dtype=np.float32)
    gt = np.asarray(Y_bbvert, dtype=np.float32)

    helper = (gt.reshape(B, H, 6).sum(axis=-1) > 0.0).astype(np.float32)

    def params(t):
        a = t[:, :, 0, :]
        b = t[:, :, 1, :]
        c = ((a + b) * np.float32(0.5)).astype(np.float32)
        r = ((b - a) * np.float32(0.5)).astype(np.float32)
        return c, r

    c, r = params(pred)
    rsq = (r * r).astype(np.float32)
    cg, rg = params(gt)
    # guarded reciprocal half-width for the normalized gt test
    rg_abs = np.maximum(np.abs(rg), np.float32(1e-7))
    s = (1.0 / rg_abs).astype(np.float32)
    t = (-cg * s).astype(np.float32)

    cols = [-c[:, :, 1], -c[:, :, 2], rsq[:, :, 1], rsq[:, :, 2],
            c[:, :, 0], rsq[:, :, 0], c[:, :, 1], c[:, :, 2],
            s[:, :, 1], t[:, :, 1], s[:, :, 2], t[:, :, 2],
            s[:, :, 0], t[:, :, 0], helper, 1.0 - helper]
    sc_all = np.stack(cols, axis=-1).astype(np.float32)  # [B,H,16]

    in_maps = []
    for k in range(NCORES):
        rows = []
        scs = []
        for b in range(BPC):
            bi = BPC * k + b
            # de-interleave xyz on host: partition row = [x(640)|y(640)|z(640)]
            xyz = X_pc[bi].reshape(PPB, FPT, 9)[:, :, :3]
            rows.append(xyz.transpose(0, 2, 1).reshape(PPB, RAWF))
            scs.append(np.broadcast_to(sc_all[bi][None], (PPB, H, SCW)))
        in_maps.append({
            "xpc": np.ascontiguousarray(np.concatenate(rows, axis=0)),
            "scal": np.ascontiguousarray(
                np.concatenate(scs, axis=0).reshape(NPART, H * SCW)),
        })
    return in_maps


def _combine(results, y_bbvert_pred, Y_bbvert):
    """Host-side: partition+core reduction and final loss combine (f64)."""
    pred = np.asarray(y_bbvert_pred, dtype=np.float32)
    gt = np.asarray(Y_bbvert, dtype=np.float32)

    helper = (gt.reshape(B, H, 6).sum(axis=-1) > 0.0).astype(np.float64)

    Sp = np.zeros((B, H)); Sg = np.zeros((B, H)); Ssh = np.zeros((B, H))
    Sln = 0.0
    for k in range(NCORES):
        r = results[k]
        Sln += r["accL"].astype(np.float64).sum()
        for b in range(BPC):
            bi = BPC * k + b
            sl = slice(PPB * b, PPB * (b + 1))
            Sp[bi] = r["accP"][sl].astype(np.float64).sum(axis=0)
            Sg[bi] = r["accG"][sl].astype(np.float64).sum(axis=0)
            Ssh[bi] = r["accS"][sl].astype(np.float64).sum(axis=0)

    # undo the helper fold: Ssh = helper*S_sel + (1-helper)*N
    Ss = np.where(helper > 0, Ssh, 0.0)
    Tp = (Ss + Sg + Sp - float(N)) * 0.5

    denom_ce = helper.sum() * N
    loss_ce = -Sln / denom_ce

    den = np.where(helper > 0, Sp + Sg - Tp + 1e-6, 1.0)
    iou_all = -(Tp / den)
    loss_iou = (iou_all * helper).sum() / helper.sum()

    l2_all = ((gt.astype(np.float64) - pred.astype(np.float64)) ** 2
              ).reshape(B, H, 6).mean(axis=-1)
    l2_pos = (l2_all * helper).sum() / helper.sum()
    negw = (1.0 - helper)[:, :, None]
    dneg = (pred[:, :, 0, :].astype(np.float64) - pred[:, :, 1, :].astype(np.float64))
    l2_neg = ((negw * dneg) ** 2).sum() / ((1.0 - helper).sum() + 1e-8)
    loss_l2 = l2_pos + l2_neg

    total = loss_ce + loss_l2 + loss_iou
    return (np.float32(total), np.float32(loss_l2),
            np.float32(loss_ce), np.float32(loss_iou))


def run(X_pc, y_bbvert_pred, Y_bbvert, trace=False):
    from concourse.bass_utils import run_bass_kernel_spmd

    nc = _get_module()
    in_maps = _make_inputs(X_pc, y_bbvert_pred, Y_bbvert)
    res = run_bass_kernel_spmd(nc, in_maps, core_ids=list(range(NCORES)),
                               trace=trace)
    out = _combine(res.results, y_bbvert_pred, Y_bbvert)
    return out, res


def kernel(X_pc, y_bbvert_pred, Y_bbvert):
    out, _ = run(X_pc, y_bbvert_pred, Y_bbvert, trace=False)
    return out


# revision 17
# speedup vs baseline: 1.0126x; 1.0126x over previous
"""Trainium2 Bass kernel for nn_BbVertLoss (point-in-bbox CE + IoU + L2 loss).

Strategy (pure data parallel, hardcoded for B=16, N=40960, H=24, 8 cores):
  - Each core gets 2 batches. Points live on partitions: partition p holds
    640 consecutive points of batch p//64 (local), laid out [128, 640*9].
  - Pred leg (real units, clip needs them):
      u_d = r_d^2 - (x_d-c_d)^2;  tcl = clip(max_d((x_d-c_d)^2 - r_d^2), +-.2)
      p = sigmoid(-100*tcl), accum -> S_p
  - GT leg (normalized units, only the sign matters):
      q_d = ((x_d-c'_d)/r'_d)^2 computed as Square(x*s + t) on ACT
      (per-partition scale AP!), combined with a stock bf16 TT MAX (2x mode),
      then one fused DVE op: g = (max(sq(x*s_x+t_x), m_yz) < 1), accum -> S_g
  - sel' = helper*|p + g - 1| + (1-helper)  (helper folded in so the Ln
    accumulation can batch across h in one group-wide instruction whose
    single accumulator directly yields the helper-weighted CE partial).
  - Ln per group of GRP h: one ACT op over [128, GRP*640], accum -> CE.
  - Host: partition+core reduction and final combine in f64.
  - Custom fused DVE ops registered into concourse.dve_ops:
      ANT_SUB2MAX:     max(in0-s0, in1-s1)
      ANT_SQMAXCLIP:   clip(max((in0-s0)^2-s1, in1), imm2, -imm2)
      ANT_SQMAXLT1SUM: (max((in0*s0+s1)^2, in1) < 1) [+ sum]
      ANT_SELHSUM:     s0*|in0 + (in1-1)| + s1 [+ sum]
      ANT_SQSUB / ANT_SQSUBMAX: DVE-route pred legs for engine balance
"""

import numpy as np

B, N, H = 16, 40960, 24
NCORES = 8
BPC = B // NCORES            # batches per core = 2
PPB = 64                     # partitions per batch
FPT = N // PPB               # points per partition = 640
NPART = BPC * PPB            # 128
RAWF = FPT * 3               # xyz de-interleaved on host: [x|y|z] per partition
import os
GRP = int(os.environ.get("KGRP", "8"))   # h-group size (ACT table amortization)
WBUFS = int(os.environ.get("KWBUFS", "4"))
NB = int(os.environ.get("KNB", "10"))     # of 24 h: pred legs on DVE route
SCW = 16                     # scal columns per (batch, h)
NGRP = (H + GRP - 1) // GRP

_CACHE = {}


def _register_custom_ops():
    """Register fused DVE ops in the module-level registries (idempotent)."""
    import concourse.dve_ops as dops
    from concourse.dve_spec import (Spec, Src0, Src1, C0, C1, C2, Zero, One,
                                    maxx, minn, sq, lower, AluOp)
    from concourse.dve_table_gen import dve_ver_for
    from concourse.dve_uop import DveOpSpec

    if "ANT_SUB2MAX" in dops._SUB_OPCODE_FOR_NAME:
        _CACHE["ops"] = {o.name: o for o in dops.OPS}
        return

    ver = dve_ver_for("TRN2")

    def ref_sub2max(in0, in1, s0, s1, imm2):
        return np.maximum(in0 - s0, in1 - s1)

    def ref_sqmaxclip(in0, in1, s0, s1, imm2):
        return np.minimum(np.maximum(np.maximum((in0 - s0) ** 2 - s1, in1),
                                     imm2), -imm2)

    def ref_sqmaxlt1sum(in0, in1, s0, s1, imm2):
        b = (np.maximum((in0 * s0 + s1) ** 2, in1) < 1.0).astype(np.float32)
        return b, b.reshape(b.shape[0], -1).sum(axis=-1, keepdims=True).astype(
            np.float32)

    def ref_selhsum(in0, in1, s0, s1, imm2):
        t = ((in1 - np.float32(1.0)) + in0).astype(np.float32)
        b = (s0 * np.abs(t) + s1).astype(np.float32)
        return b, b.reshape(b.shape[0], -1).sum(axis=-1, keepdims=True).astype(
            np.float32)

    def ref_sqsubmax(in0, in1, s0, s1, imm2):
        return np.maximum((in0 - s0) ** 2 - s1, in1)

    _t = Src0 + (Src1 - One)
    _q = sq(Src0 * C0 + C1)
    specs = [
        ("ANT_SUB2MAX", Spec(body=maxx(Src0 - C0, Src1 - C1),
                             reference=ref_sub2max)),
        ("ANT_SQMAXCLIP", Spec(body=minn(maxx(maxx(sq(Src0 - C0) - C1, Src1),
                                              C2), Zero - C2),
                               reference=ref_sqmaxclip)),
        ("ANT_SQMAXLT1SUM", Spec(body=(maxx(_q, Src1) < One),
                                 accum=AluOp.ADD, reference=ref_sqmaxlt1sum)),
        ("ANT_SELHSUM", Spec(body=maxx(_t, Zero - _t) * C0 + C1,
                             accum=AluOp.ADD, reference=ref_selhsum)),
        ("ANT_SQSUBMAX", Spec(body=maxx(sq(Src0 - C0) - C1, Src1),
                              reference=ref_sqsubmax)),
        ("ANT_SQSUB", Spec(body=sq(Src0 - C0) - C1,
                           reference=lambda in0, in1, s0, s1, imm2:
                               (in0 - s0) ** 2 - s1)),
    ]
    for name, spec in specs:
        opcode = max(dops._SUB_OPCODE_FOR_NAME.values()) + 1
        assert opcode < 0x20
        tmp = DveOpSpec(name=name, opcode=opcode, uops=lower(spec, ver=ver),
                        rd1_en=True)
        op = dops.DveOp(name, spec, subdim=False, uops_sha={ver: tmp.sha(ver)})
        dops.OPS.append(op)
        dops.CUSTOM_DVE_SPECS[name] = spec
        dops._SUB_OPCODE_FOR_NAME[name] = opcode
    _CACHE["ops"] = {o.name: o for o in dops.OPS}


def _build_module():
    import concourse.bacc as bacc
    import concourse.tile as tile
    from concourse import mybir

    _register_custom_ops()
    OPS = _CACHE["ops"]

    f32 = mybir.dt.float32
    bf16 = mybir.dt.bfloat16
    fp16 = mybir.dt.float16
    Act = mybir.ActivationFunctionType
    Alu = mybir.AluOpType

    # pred h's routed to all-DVE legs: the whole LAST group goes DVE so the
    # Scalar tail (last sigmoids + ln) shrinks while Vector's end-idle absorbs
    # it; the remainder spreads over the earlier h's.
    dve_hs = set(int(round(i * H / max(NB, 1))) for i in range(NB)) if NB else set()

    nc = bacc.Bacc("TRN2", debug=False)

    xpc = nc.dram_tensor("xpc", [NPART, RAWF], f32, kind="ExternalInput")
    scal = nc.dram_tensor("scal", [NPART, H * SCW], f32, kind="ExternalInput")
    accP_d = nc.dram_tensor("accP", [NPART, H], f32, kind="ExternalOutput")
    accG_d = nc.dram_tensor("accG", [NPART, H], f32, kind="ExternalOutput")
    accS_d = nc.dram_tensor("accS", [NPART, H], f32, kind="ExternalOutput")
    accL_d = nc.dram_tensor("accL", [NPART, 2 * NGRP], f32, kind="ExternalOutput")

    with tile.TileContext(nc) as tc:
        with (
            tc.tile_pool(name="data", bufs=1) as data,
            tc.tile_pool(name="work", bufs=WBUFS) as work,
            tc.tile_pool(name="phase", bufs=GRP + 3) as phase,
            tc.tile_pool(name="selp", bufs=2) as selp,
            tc.tile_pool(name="pp", bufs=4, space="PSUM") as pp,
        ):
            sc = data.tile([NPART, H * SCW], f32, tag="sc")
            nc.sync.dma_start(out=sc[:], in_=scal[:])
            raw = data.tile([NPART, RAWF], f32, tag="raw")
            for dord in (1, 2, 0):   # y first (consumed first), x last
                cs = slice(FPT * dord, FPT * (dord + 1))
                for half in range(2):
                    p0, p1 = 64 * half, 64 * (half + 1)
                    nc.sync.dma_start(out=raw[p0:p1, cs], in_=xpc[p0:p1, cs])
            eps8 = data.tile([NPART, 1], f32, tag="eps8")
            nc.vector.memset(eps8[:], 1e-8)
            warm = data.tile([NPART, 1], f32, tag="warm")
            nc.scalar.activation(warm[:], eps8[:], Act.Sigmoid,
                                 bias=0.0, scale=-100.0)

            accP = data.tile([NPART, H], f32, tag="accP")
            accG = data.tile([NPART, H], f32, tag="accG")
            accS = data.tile([NPART, H], f32, tag="accS")
            accL = data.tile([NPART, 2 * NGRP], f32, tag="accL")

            xs = [raw[:, FPT * d : FPT * (d + 1)] for d in range(3)]

            def col(h, j):
                return sc[:, SCW * h + j : SCW * h + j + 1]

            gsizes = [int(v) for v in os.environ.get(
                "KGL", "8,8,8").split(",")]
            assert sum(gsizes) == H
            gstarts = [sum(gsizes[:i]) for i in range(len(gsizes))]
            for gi, (h0, ng) in enumerate(zip(gstarts, gsizes)):
                hs = range(h0, h0 + ng)
                selbig = selp.tile([NPART, ng * FPT], bf16, tag="selbig")
                for h in hs:
                    # pred leg: tcl = clip(max_d((x_d-c_d)^2 - r_d^2), +-0.2)
                    if h in dve_hs:
                        qsy = work.tile([NPART, FPT], f32, tag="qsy")
                        nc.vector._custom_dve(OPS["ANT_SQSUB"], out=qsy[:],
                                              in0=xs[1],
                                              s0=col(h, 6), s1=col(h, 2))
                        m1 = work.tile([NPART, FPT], f32, tag="m1")
                        nc.vector._custom_dve(OPS["ANT_SQSUBMAX"], out=m1[:],
                                              in0=xs[2], in1=qsy[:],
                                              s0=col(h, 7), s1=col(h, 3))
                    else:
                        sqy = work.tile([NPART, FPT], f32, tag="sqy")
                        nc.scalar.activation(sqy[:], xs[1], Act.Square,
                                             bias=col(h, 0), scale=1.0)
                        sqz = work.tile([NPART, FPT], f32, tag="sqz")
                        nc.scalar.activation(sqz[:], xs[2], Act.Square,
                                             bias=col(h, 1), scale=1.0)
                        m1 = work.tile([NPART, FPT], f32, tag="m1")
                        nc.vector._custom_dve(OPS["ANT_SUB2MAX"], out=m1[:],
                                              in0=sqy[:], in1=sqz[:],
                                              s0=col(h, 2), s1=col(h, 3))
                    tcl = phase.tile([NPART, FPT], f32, tag="tcl")
                    nc.vector._custom_dve(OPS["ANT_SQMAXCLIP"], out=tcl[:],
                                          in0=xs[0], in1=m1[:],
                                          s0=col(h, 4), s1=col(h, 5), imm2=-0.2)

                    # gt leg, normalized: q_d = Square(x*s+t); in-box = all q<1
                    qgy = work.tile([NPART, FPT], bf16, tag="qgy")
                    nc.scalar.activation(qgy[:], xs[1], Act.Square,
                                         bias=col(h, 9), scale=col(h, 8))
                    qgz = work.tile([NPART, FPT], bf16, tag="qgz")
                    nc.scalar.activation(qgz[:], xs[2], Act.Square,
                                         bias=col(h, 11), scale=col(h, 10))
                    myz = work.tile([NPART, FPT], bf16, tag="myz")
                    nc.vector.tensor_tensor(out=myz[:], in0=qgy[:],
                                            in1=qgz[:], op=Alu.max)
                    g = phase.tile([NPART, FPT], bf16, tag="g")
                    nc.vector._custom_dve(OPS["ANT_SQMAXLT1SUM"], out=g[:],
                                          in0=xs[0], in1=myz[:],
                                          s0=col(h, 12), s1=col(h, 13),
                                          accum_out=accG[:, h : h + 1])
                    # tail fused per h: Square and Sigmoid share a
                    # table set, so interleaving costs no table loads and
                    # lets sel' tiles finish progressively before the ln
                    p = pp.tile([NPART, FPT], f32, tag="p")
                    nc.scalar.activation(p[:], tcl[:], Act.Sigmoid,
                                         bias=0.0, scale=-100.0,
                                         accum_out=accP[:, h : h + 1])
                    j = h - h0
                    nc.vector._custom_dve(
                        OPS["ANT_SELHSUM"],
                        out=selbig[:, j * FPT : (j + 1) * FPT],
                        in0=p[:], in1=g[:],
                        s0=col(h, 14), s1=col(h, 15),
                        accum_out=accS[:, h : h + 1])
                # one batched Ln for the whole group; its accumulator IS the
                # helper-weighted CE partial sum for these h
                nc.scalar.activation(selbig[:], selbig[:], Act.Ln,
                                     bias=eps8[:], scale=1.0,
                                     accum_out=accL[:, 2 * gi : 2 * gi + 1])

            nc.sync.dma_start(out=accP_d[:], in_=accP[:])
            nc.sync.dma_start(out=accG_d[:], in_=accG[:])
            nc.sync.dma_start(out=accS_d[:], in_=accS[:])
            nc.sync.dma_start(out=accL_d[:], in_=accL[:])

    nc.compile()
    return nc


def _get_module():
    if "nc" not in _CACHE:
        _CACHE["nc"] = _build_module()
    return _CACHE["nc"]


def _make_inputs(X_pc, y_bbvert_pred, Y_bbvert):
    """Build per-core input maps (host-side shard + scalar precompute)."""
    X_pc = np.ascontiguousarray(X_pc, dtype=np.float32)
    pred = np.asarray(y_bbvert_pred, dtype=np.float32)
    gt = np.asarray(Y_bbvert, dtype=np.float32)

    helper = (gt.reshape(B, H, 6).sum(axis=-1) > 0.0).astype(np.float32)

    def params(t):
        a = t[:, :, 0, :]
        b = t[:, :, 1, :]
        c = ((a + b) * np.float32(0.5)).astype(np.float32)
        r = ((b - a) * np.float32(0.5)).astype(np.float32)
        return c, r

    c, r = params(pred)
    rsq = (r * r).astype(np.float32)
    cg, rg = params(gt)
    # guarded reciprocal half-width for the normalized gt test
    rg_abs = np.maximum(np.abs(rg), np.float32(1e-7))
    s = (1.0 / rg_abs).astype(np.float32)
    t = (-cg * s).astype(np.float32)

    cols = [-c[:, :, 1], -c[:, :, 2], rsq[:, :, 1], rsq[:, :, 2],
            c[:, :, 0], rsq[:, :, 0], c[:, :, 1], c[:, :, 2],
            s[:, :, 1], t[:, :, 1], s[:, :, 2], t[:, :, 2],
            s[:, :, 0], t[:, :, 0], helper, 1.0 - helper]
    sc_all = np.stack(cols, axis=-1).astype(np.float32)  # [B,H,16]

    in_maps = []
    for k in range(NCORES):
        rows = []
        scs = []
        for b in range(BPC):
            bi = BPC * k + b
            # de-interleave xyz on host: partition row = [x(640)|y(640)|z(640)]
            xyz = X_pc[bi].reshape(PPB, FPT, 9)[:, :, :3]
            rows.append(xyz.transpose(0, 2, 1).reshape(PPB, RAWF))
            scs.append(np.broadcast_to(sc_all[bi][None], (PPB, H, SCW)))
        in_maps.append({
            "xpc": np.ascontiguousarray(np.concatenate(rows, axis=0)),
            "scal": np.ascontiguousarray(
                np.concatenate(scs, axis=0).reshape(NPART, H * SCW)),
        })
    return in_maps


def _combine(results, y_bbvert_pred, Y_bbvert):
    """Host-side: partition+core reduction and final loss combine (f64)."""
    pred = np.asarray(y_bbvert_pred, dtype=np.float32)
    gt = np.asarray(Y_bbvert, dtype=np.float32)

    helper = (gt.reshape(B, H, 6).sum(axis=-1) > 0.0).astype(np.float64)

    Sp = np.zeros((B, H)); Sg = np.zeros((B, H)); Ssh = np.zeros((B, H))
    Sln = 0.0
    for k in range(NCORES):
        r = results[k]
        Sln += r["accL"].astype(np.float64).sum()
        for b in range(BPC):
            bi = BPC * k + b
            sl = slice(PPB * b, PPB * (b + 1))
            Sp[bi] = r["accP"][sl].astype(np.float64).sum(axis=0)
            Sg[bi] = r["accG"][sl].astype(np.float64).sum(axis=0)
            Ssh[bi] = r["accS"][sl].astype(np.float64).sum(axis=0)

    # undo the helper fold: Ssh = helper*S_sel + (1-helper)*N
    Ss = np.where(helper > 0, Ssh, 0.0)
    Tp = (Ss + Sg + Sp - float(N)) * 0.5

    denom_ce = helper.sum() * N
    loss_ce = -Sln / denom_ce

    den = np.where(helper > 0, Sp + Sg - Tp + 1e-6, 1.0)
    iou_all = -(Tp / den)
    loss_iou = (iou_all * helper).sum() / helper.sum()

    l2_all = ((gt.astype(np.float64) - pred.astype(np.float64)) ** 2
              ).reshape(B, H, 6).mean(axis=-1)
    l2_pos = (l2_all * helper).sum() / helper.sum()
    negw = (1.0 - helper)[:, :, None]
    dneg = (pred[:, :, 0, :].astype(np.float64) - pred[:, :, 1, :].astype(np.float64))
    l2_neg = ((negw * dneg) ** 2).sum() / ((1.0 - helper).sum() + 1e-8)
    loss_l2 = l2_pos + l2_neg

    total = loss_ce + loss_l2 + loss_iou
    return (np.float32(total), np.float32(loss_l2),
            np.float32(loss_ce), np.float32(loss_iou))


def run(X_pc, y_bbvert_pred, Y_bbvert, trace=False):
    from concourse.bass_utils import run_bass_kernel_spmd

    nc = _get_module()
    in_maps = _make_inputs(X_pc, y_bbvert_pred, Y_bbvert)
    res = run_bass_kernel_spmd(nc, in_maps, core_ids=list(range(NCORES)),
                               trace=trace)
    out = _combine(res.results, y_bbvert_pred, Y_bbvert)
    return out, res


def kernel(X_pc, y_bbvert_pred, Y_bbvert):
    out, _ = run(X_pc, y_bbvert_pred, Y_bbvert, trace=False)
    return out


# revision 18
# speedup vs baseline: 1.0267x; 1.0139x over previous
"""Trainium2 Bass kernel for nn_BbVertLoss (point-in-bbox CE + IoU + L2 loss).

Strategy (pure data parallel, hardcoded for B=16, N=40960, H=24, 8 cores):
  - Each core gets 2 batches. Points live on partitions: partition p holds
    640 consecutive points of batch p//64 (local), laid out [128, 640*9].
  - Pred leg (real units, clip needs them):
      u_d = r_d^2 - (x_d-c_d)^2;  tcl = clip(max_d((x_d-c_d)^2 - r_d^2), +-.2)
      p = sigmoid(-100*tcl), accum -> S_p
  - GT leg (normalized units, only the sign matters):
      q_d = ((x_d-c'_d)/r'_d)^2 computed as Square(x*s + t) on ACT
      (per-partition scale AP!), combined with a stock bf16 TT MAX (2x mode),
      then one fused DVE op: g = (max(sq(x*s_x+t_x), m_yz) < 1), accum -> S_g
  - sel' = helper*|p + g - 1| + (1-helper)  (helper folded in so the Ln
    accumulation can batch across h in one group-wide instruction whose
    single accumulator directly yields the helper-weighted CE partial).
  - Ln per group of GRP h: one ACT op over [128, GRP*640], accum -> CE.
  - Host: partition+core reduction and final combine in f64.
  - Custom fused DVE ops registered into concourse.dve_ops:
      ANT_SUB2MAX:     max(in0-s0, in1-s1)
      ANT_SQMAXCLIP:   clip(max((in0-s0)^2-s1, in1), imm2, -imm2)
      ANT_SQMAXLT1SUM: (max((in0*s0+s1)^2, in1) < 1) [+ sum]
      ANT_SELHSUM:     s0*|in0 + (in1-1)| + s1 [+ sum]
      ANT_SQSUB / ANT_SQSUBMAX: DVE-route pred legs for engine balance
"""

import numpy as np

B, N, H = 16, 40960, 24
NCORES = 8
BPC = B // NCORES            # batches per core = 2
PPB = 64                     # partitions per batch
FPT = N // PPB               # points per partition = 640
NPART = BPC * PPB            # 128
RAWF = FPT * 3               # xyz de-interleaved on host: [x|y|z] per partition
import os
GRP = int(os.environ.get("KGRP", "8"))   # h-group size (ACT table amortization)
WBUFS = int(os.environ.get("KWBUFS", "4"))
NB = int(os.environ.get("KNB", "12"))     # of 24 h: pred legs on DVE route
SCW = 16                     # scal columns per (batch, h)
NGRP = (H + GRP - 1) // GRP

_CACHE = {}


def _register_custom_ops():
    """Register fused DVE ops in the module-level registries (idempotent)."""
    import concourse.dve_ops as dops
    from concourse.dve_spec import (Spec, Src0, Src1, C0, C1, C2, Zero, One,
                                    maxx, minn, sq, lower, AluOp)
    from concourse.dve_table_gen import dve_ver_for
    from concourse.dve_uop import DveOpSpec

    if "ANT_SUB2MAX" in dops._SUB_OPCODE_FOR_NAME:
        _CACHE["ops"] = {o.name: o for o in dops.OPS}
        return

    ver = dve_ver_for("TRN2")

    def ref_sub2max(in0, in1, s0, s1, imm2):
        return np.maximum(in0 - s0, in1 - s1)

    def ref_sqmaxclip(in0, in1, s0, s1, imm2):
        return np.minimum(np.maximum(np.maximum((in0 - s0) ** 2 - s1, in1),
                                     imm2), -imm2)

    def ref_sqmaxlt1sum(in0, in1, s0, s1, imm2):
        b = (np.maximum((in0 * s0 + s1) ** 2, in1) < 1.0).astype(np.float32)
        return b, b.reshape(b.shape[0], -1).sum(axis=-1, keepdims=True).astype(
            np.float32)

    def ref_selhsum(in0, in1, s0, s1, imm2):
        t = ((in1 - np.float32(1.0)) + in0).astype(np.float32)
        b = (s0 * np.abs(t) + s1).astype(np.float32)
        return b, b.reshape(b.shape[0], -1).sum(axis=-1, keepdims=True).astype(
            np.float32)

    def ref_sqsubmax(in0, in1, s0, s1, imm2):
        return np.maximum((in0 - s0) ** 2 - s1, in1)

    _t = Src0 + (Src1 - One)
    _q = sq(Src0 * C0 + C1)
    specs = [
        ("ANT_SUB2MAX", Spec(body=maxx(Src0 - C0, Src1 - C1),
                             reference=ref_sub2max)),
        ("ANT_SQMAXCLIP", Spec(body=minn(maxx(maxx(sq(Src0 - C0) - C1, Src1),
                                              C2), Zero - C2),
                               reference=ref_sqmaxclip)),
        ("ANT_SQMAXLT1SUM", Spec(body=(maxx(_q, Src1) < One),
                                 accum=AluOp.ADD, reference=ref_sqmaxlt1sum)),
        ("ANT_SELHSUM", Spec(body=maxx(_t, Zero - _t) * C0 + C1,
                             accum=AluOp.ADD, reference=ref_selhsum)),
        ("ANT_SQSUBMAX", Spec(body=maxx(sq(Src0 - C0) - C1, Src1),
                              reference=ref_sqsubmax)),
        ("ANT_SQSUB", Spec(body=sq(Src0 - C0) - C1,
                           reference=lambda in0, in1, s0, s1, imm2:
                               (in0 - s0) ** 2 - s1)),
    ]
    for name, spec in specs:
        opcode = max(dops._SUB_OPCODE_FOR_NAME.values()) + 1
        assert opcode < 0x20
        tmp = DveOpSpec(name=name, opcode=opcode, uops=lower(spec, ver=ver),
                        rd1_en=True)
        op = dops.DveOp(name, spec, subdim=False, uops_sha={ver: tmp.sha(ver)})
        dops.OPS.append(op)
        dops.CUSTOM_DVE_SPECS[name] = spec
        dops._SUB_OPCODE_FOR_NAME[name] = opcode
    _CACHE["ops"] = {o.name: o for o in dops.OPS}


def _build_module():
    import concourse.bacc as bacc
    import concourse.tile as tile
    from concourse import mybir

    _register_custom_ops()
    OPS = _CACHE["ops"]

    f32 = mybir.dt.float32
    bf16 = mybir.dt.bfloat16
    fp16 = mybir.dt.float16
    Act = mybir.ActivationFunctionType
    Alu = mybir.AluOpType

    # pred h's routed to all-DVE legs: the whole LAST group goes DVE so the
    # Scalar tail (last sigmoids + ln) shrinks while Vector's end-idle absorbs
    # it; the remainder spreads over the earlier h's.
    dve_hs = set(int(round(i * H / max(NB, 1))) for i in range(NB)) if NB else set()

    nc = bacc.Bacc("TRN2", debug=False)

    xpc = nc.dram_tensor("xpc", [NPART, RAWF], f32, kind="ExternalInput")
    scal = nc.dram_tensor("scal", [NPART, H * SCW], f32, kind="ExternalInput")
    accP_d = nc.dram_tensor("accP", [NPART, H], f32, kind="ExternalOutput")
    accG_d = nc.dram_tensor("accG", [NPART, H], f32, kind="ExternalOutput")
    accS_d = nc.dram_tensor("accS", [NPART, H], f32, kind="ExternalOutput")
    accL_d = nc.dram_tensor("accL", [NPART, 2 * NGRP], f32, kind="ExternalOutput")

    with tile.TileContext(nc) as tc:
        with (
            tc.tile_pool(name="data", bufs=1) as data,
            tc.tile_pool(name="work", bufs=WBUFS) as work,
            tc.tile_pool(name="phase", bufs=GRP + 3) as phase,
            tc.tile_pool(name="selp", bufs=2) as selp,
            tc.tile_pool(name="pp", bufs=4, space="PSUM") as pp,
        ):
            sc = data.tile([NPART, H * SCW], f32, tag="sc")
            nc.sync.dma_start(out=sc[:], in_=scal[:])
            raw = data.tile([NPART, RAWF], f32, tag="raw")
            for dord in (1, 2, 0):   # y first (consumed first), x last
                cs = slice(FPT * dord, FPT * (dord + 1))
                for half in range(2):
                    p0, p1 = 64 * half, 64 * (half + 1)
                    nc.sync.dma_start(out=raw[p0:p1, cs], in_=xpc[p0:p1, cs])
            eps8 = data.tile([NPART, 1], f32, tag="eps8")
            nc.vector.memset(eps8[:], 1e-8)
            warm = data.tile([NPART, 1], f32, tag="warm")
            nc.scalar.activation(warm[:], eps8[:], Act.Sigmoid,
                                 bias=0.0, scale=-100.0)

            accP = data.tile([NPART, H], f32, tag="accP")
            accG = data.tile([NPART, H], f32, tag="accG")
            accS = data.tile([NPART, H], f32, tag="accS")
            accL = data.tile([NPART, 2 * NGRP], f32, tag="accL")

            xs = [raw[:, FPT * d : FPT * (d + 1)] for d in range(3)]

            def col(h, j):
                return sc[:, SCW * h + j : SCW * h + j + 1]

            gsizes = [int(v) for v in os.environ.get(
                "KGL", "8,8,8").split(",")]
            assert sum(gsizes) == H
            gstarts = [sum(gsizes[:i]) for i in range(len(gsizes))]
            for gi, (h0, ng) in enumerate(zip(gstarts, gsizes)):
                hs = range(h0, h0 + ng)
                selbig = selp.tile([NPART, ng * FPT], bf16, tag="selbig")
                for h in hs:
                    # pred leg: tcl = clip(max_d((x_d-c_d)^2 - r_d^2), +-0.2)
                    if h in dve_hs:
                        qsy = work.tile([NPART, FPT], f32, tag="qsy")
                        nc.vector._custom_dve(OPS["ANT_SQSUB"], out=qsy[:],
                                              in0=xs[1],
                                              s0=col(h, 6), s1=col(h, 2))
                        m1 = work.tile([NPART, FPT], f32, tag="m1")
                        nc.vector._custom_dve(OPS["ANT_SQSUBMAX"], out=m1[:],
                                              in0=xs[2], in1=qsy[:],
                                              s0=col(h, 7), s1=col(h, 3))
                    else:
                        sqy = work.tile([NPART, FPT], f32, tag="sqy")
                        nc.scalar.activation(sqy[:], xs[1], Act.Square,
                                             bias=col(h, 0), scale=1.0)
                        sqz = work.tile([NPART, FPT], f32, tag="sqz")
                        nc.scalar.activation(sqz[:], xs[2], Act.Square,
                                             bias=col(h, 1), scale=1.0)
                        m1 = work.tile([NPART, FPT], f32, tag="m1")
                        nc.vector._custom_dve(OPS["ANT_SUB2MAX"], out=m1[:],
                                              in0=sqy[:], in1=sqz[:],
                                              s0=col(h, 2), s1=col(h, 3))
                    tcl = phase.tile([NPART, FPT], f32, tag="tcl")
                    nc.vector._custom_dve(OPS["ANT_SQMAXCLIP"], out=tcl[:],
                                          in0=xs[0], in1=m1[:],
                                          s0=col(h, 4), s1=col(h, 5), imm2=-0.2)

                    # gt leg, normalized: q_d = Square(x*s+t); in-box = all q<1
                    qgy = work.tile([NPART, FPT], bf16, tag="qgy")
                    nc.scalar.activation(qgy[:], xs[1], Act.Square,
                                         bias=col(h, 9), scale=col(h, 8))
                    qgz = work.tile([NPART, FPT], bf16, tag="qgz")
                    nc.scalar.activation(qgz[:], xs[2], Act.Square,
                                         bias=col(h, 11), scale=col(h, 10))
                    myz = work.tile([NPART, FPT], bf16, tag="myz")
                    nc.vector.tensor_tensor(out=myz[:], in0=qgy[:],
                                            in1=qgz[:], op=Alu.max)
                    g = phase.tile([NPART, FPT], bf16, tag="g")
                    nc.vector._custom_dve(OPS["ANT_SQMAXLT1SUM"], out=g[:],
                                          in0=xs[0], in1=myz[:],
                                          s0=col(h, 12), s1=col(h, 13),
                                          accum_out=accG[:, h : h + 1])
                    # tail fused per h: Square and Sigmoid share a
                    # table set, so interleaving costs no table loads and
                    # lets sel' tiles finish progressively before the ln
                    p = pp.tile([NPART, FPT], f32, tag="p")
                    nc.scalar.activation(p[:], tcl[:], Act.Sigmoid,
                                         bias=0.0, scale=-100.0,
                                         accum_out=accP[:, h : h + 1])
                    j = h - h0
                    nc.vector._custom_dve(
                        OPS["ANT_SELHSUM"],
                        out=selbig[:, j * FPT : (j + 1) * FPT],
                        in0=p[:], in1=g[:],
                        s0=col(h, 14), s1=col(h, 15),
                        accum_out=accS[:, h : h + 1])
                # one batched Ln for the whole group; its accumulator IS the
                # helper-weighted CE partial sum for these h
                nc.scalar.activation(selbig[:], selbig[:], Act.Ln,
                                     bias=eps8[:], scale=1.0,
                                     accum_out=accL[:, 2 * gi : 2 * gi + 1])

            nc.sync.dma_start(out=accP_d[:], in_=accP[:])
            nc.sync.dma_start(out=accG_d[:], in_=accG[:])
            nc.sync.dma_start(out=accS_d[:], in_=accS[:])
            nc.sync.dma_start(out=accL_d[:], in_=accL[:])

    nc.compile()
    return nc


def _get_module():
    if "nc" not in _CACHE:
        _CACHE["nc"] = _build_module()
    return _CACHE["nc"]


def _make_inputs(X_pc, y_bbvert_pred, Y_bbvert):
    """Build per-core input maps (host-side shard + scalar precompute)."""
    X_pc = np.ascontiguousarray(X_pc, dtype=np.float32)
    pred = np.asarray(y_bbvert_pred, dtype=np.float32)
    gt = np.asarray(Y_bbvert, dtype=np.float32)

    helper = (gt.reshape(B, H, 6).sum(axis=-1) > 0.0).astype(np.float32)

    def params(t):
        a = t[:, :, 0, :]
        b = t[:, :, 1, :]
        c = ((a + b) * np.float32(0.5)).astype(np.float32)
        r = ((b - a) * np.float32(0.5)).astype(np.float32)
        return c, r

    c, r = params(pred)
    rsq = (r * r).astype(np.float32)
    cg, rg = params(gt)
    # guarded reciprocal half-width for the normalized gt test
    rg_abs = np.maximum(np.abs(rg), np.float32(1e-7))
    s = (1.0 / rg_abs).astype(np.float32)
    t = (-cg * s).astype(np.float32)

    cols = [-c[:, :, 1], -c[:, :, 2], rsq[:, :, 1], rsq[:, :, 2],
            c[:, :, 0], rsq[:, :, 0], c[:, :, 1], c[:, :, 2],
            s[:, :, 1], t[:, :, 1], s[:, :, 2], t[:, :, 2],
            s[:, :, 0], t[:, :, 0], helper, 1.0 - helper]
    sc_all = np.stack(cols, axis=-1).astype(np.float32)  # [B,H,16]

    in_maps = []
    for k in range(NCORES):
        rows = []
        scs = []
        for b in range(BPC):
            bi = BPC * k + b
            # de-interleave xyz on host: partition row = [x(640)|y(640)|z(640)]
            xyz = X_pc[bi].reshape(PPB, FPT, 9)[:, :, :3]
            rows.append(xyz.transpose(0, 2, 1).reshape(PPB, RAWF))
            scs.append(np.broadcast_to(sc_all[bi][None], (PPB, H, SCW)))
        in_maps.append({
            "xpc": np.ascontiguousarray(np.concatenate(rows, axis=0)),
            "scal": np.ascontiguousarray(
                np.concatenate(scs, axis=0).reshape(NPART, H * SCW)),
        })
    return in_maps


def _combine(results, y_bbvert_pred, Y_bbvert):
    """Host-side: partition+core reduction and final loss combine (f64)."""
    pred = np.asarray(y_bbvert_pred, dtype=np.float32)
    gt = np.asarray(Y_bbvert, dtype=np.float32)

    helper = (gt.reshape(B, H, 6).sum(axis=-1) > 0.0).astype(np.float64)

    Sp = np.zeros((B, H)); Sg = np.zeros((B, H)); Ssh = np.zeros((B, H))
    Sln = 0.0
    for k in range(NCORES):
        r = results[k]
        Sln += r["accL"].astype(np.float64).sum()
        for b in range(BPC):
            bi = BPC * k + b
            sl = slice(PPB * b, PPB * (b + 1))
            Sp[bi] = r["accP"][sl].astype(np.float64).sum(axis=0)
            Sg[bi] = r["accG"][sl].astype(np.float64).sum(axis=0)
            Ssh[bi] = r["accS"][sl].astype(np.float64).sum(axis=0)

    # undo the helper fold: Ssh = helper*S_sel + (1-helper)*N
    Ss = np.where(helper > 0, Ssh, 0.0)
    Tp = (Ss + Sg + Sp - float(N)) * 0.5

    denom_ce = helper.sum() * N
    loss_ce = -Sln / denom_ce

    den = np.where(helper > 0, Sp + Sg - Tp + 1e-6, 1.0)
    iou_all = -(Tp / den)
    loss_iou = (iou_all * helper).sum() / helper.sum()

    l2_all = ((gt.astype(np.float64) - pred.astype(np.float64)) ** 2
              ).reshape(B, H, 6).mean(axis=-1)
    l2_pos = (l2_all * helper).sum() / helper.sum()
    negw = (1.0 - helper)[:, :, None]
    dneg = (pred[:, :, 0, :].astype(np.float64) - pred[:, :, 1, :].astype(np.float64))
    l2_neg = ((negw * dneg) ** 2).sum() / ((1.0 - helper).sum() + 1e-8)
    loss_l2 = l2_pos + l2_neg

    total = loss_ce + loss_l2 + loss_iou
    return (np.float32(total), np.float32(loss_l2),
            np.float32(loss_ce), np.float32(loss_iou))


def run(X_pc, y_bbvert_pred, Y_bbvert, trace=False):
    from concourse.bass_utils import run_bass_kernel_spmd

    nc = _get_module()
    in_maps = _make_inputs(X_pc, y_bbvert_pred, Y_bbvert)
    res = run_bass_kernel_spmd(nc, in_maps, core_ids=list(range(NCORES)),
                               trace=trace)
    out = _combine(res.results, y_bbvert_pred, Y_bbvert)
    return out, res


def kernel(X_pc, y_bbvert_pred, Y_bbvert):
    out, _ = run(X_pc, y_bbvert_pred, Y_bbvert, trace=False)
    return out
